# revision 7
# baseline (speedup 1.0000x reference)
"""Contrastive-loss kernel for Trainium2 (8 NeuronCores, data-parallel).

Reference computation (per batch row b):
    samples = concat([positives[b, -1], negatives[b]])        # [129, 1024]
    sim[s]  = <a_b, samples[s]> / (max(|a_b|,eps) * max(|samples[s]|,eps))
    loss_b  = logsumexp(sim) - sim[0]
    loss    = mean_b loss_b

Sharding: batch dim (2048) split across 8 cores (256 rows each).
Per core layout: batch on SBUF partitions (2 groups of 128), D on free dim.
Hot loop per sample index s (one [128, 1024] pass each):
    DVE scalar_tensor_tensor: out = samp*anchor, accum -> dot[:, s]
    ACT activation(Square):   out = samp^2,      accum -> ns2[:, s]
Negatives stream in [128, 4, 1024] 2 MiB chunks (contiguous per partition).
accum_out writes are NOT tracked by the Tile scheduler (observed races on
HW), so explicit add_dep_helper edges order them before their readers.
The [128, 129] softmax epilogue runs on-chip; each core outputs 128x2
per-row losses, summed/averaged on host (the allreduce-mean equivalent).
"""

import numpy as np
from contextlib import ExitStack

import concourse.bass as bass
import concourse.tile as tile
import concourse.mybir as mybir
from concourse import bacc
from concourse.bass_utils import run_bass_kernel_spmd
from concourse.tile_rust import add_dep_helper

F32 = mybir.dt.float32
ALU = mybir.AluOpType
AF = mybir.ActivationFunctionType

N_CORES = 8
B = 2048
B_LOC = B // N_CORES          # 256
D = 1024
N_NEG = 128
S = N_NEG + 1                 # 129 logits per row; s=0 is the positive
EPS = 1e-6
S_CHUNK = 4                   # sample-indices per DMA (2 MiB transfers)


def _build_nc(
    b_loc: int = B_LOC,
    n_neg: int = N_NEG,
    s_chunk: int = S_CHUNK,
    bulk_bufs: int = 6,
) -> bass.Bass:
    n_groups = b_loc // 128
    s_tot = n_neg + 1
    nc = bacc.Bacc("TRN2", target_bir_lowering=False)
    anchor = nc.dram_tensor("anchor", [b_loc, D], F32, kind="ExternalInput")
    pos = nc.dram_tensor("pos", [b_loc, D], F32, kind="ExternalInput")
    neg = nc.dram_tensor("neg", [b_loc, n_neg, D], F32, kind="ExternalInput")
    out = nc.dram_tensor("loss_cols", [128, n_groups], F32, kind="ExternalOutput")

    with tile.TileContext(nc) as tc, ExitStack() as ctx:
        bulk = ctx.enter_context(tc.tile_pool(name="bulk", bufs=bulk_bufs))
        apool = ctx.enter_context(tc.tile_pool(name="apool", bufs=2))
        tscr = ctx.enter_context(tc.tile_pool(name="tscr", bufs=2))
        ascr = ctx.enter_context(tc.tile_pool(name="ascr", bufs=2))
        persist = ctx.enter_context(tc.tile_pool(name="persist", bufs=1))
        sm = ctx.enter_context(tc.tile_pool(name="sm", bufs=1))

        def dot_accum(samp_ap, a_tile, accum_col):
            sv = tscr.tile([128, D], F32, tag="tscr")
            return nc.vector.scalar_tensor_tensor(
                out=sv[:], in0=samp_ap, scalar=1.0, in1=a_tile[:],
                op0=ALU.mult, op1=ALU.mult, accum_out=accum_col,
            )

        def sq_accum(samp_ap, accum_col):
            sa = ascr.tile([128, D], F32, tag="ascr")
            return nc.scalar.activation(
                out=sa[:], in_=samp_ap, func=AF.Square, accum_out=accum_col
            )

        dots, ns2s, na2s = [], [], []
        dot_prods, ns2_prods, na2_prods = [], [], []
        for g in range(n_groups):
            b0 = g * 128
            a_tile = apool.tile([128, D], F32, tag="a_tile")
            nc.sync.dma_start(out=a_tile[:], in_=anchor[b0 : b0 + 128, :])

            dot_all = persist.tile([128, s_tot], F32, tag=f"dot{g}")
            ns2_all = persist.tile([128, s_tot], F32, tag=f"ns2{g}")
            na2 = persist.tile([128, 1], F32, tag=f"na2{g}")
            dps, nps = [], []

            na2_prods.append(sq_accum(a_tile[:], na2[:]))

            # s = 0: the (last) positive
            p_tile = apool.tile([128, D], F32, tag="p_tile")
            nc.sync.dma_start(out=p_tile[:], in_=pos[b0 : b0 + 128, :])
            dps.append(dot_accum(p_tile[:], a_tile, dot_all[:, 0:1]))
            nps.append(sq_accum(p_tile[:], ns2_all[:, 0:1]))

            # s = 1..n_neg: negatives, streamed in chunks of s_chunk
            for c in range(n_neg // s_chunk):
                nt = bulk.tile([128, s_chunk, D], F32, tag="negchunk")
                nc.sync.dma_start(
                    out=nt[:],
                    in_=neg[b0 : b0 + 128, c * s_chunk : (c + 1) * s_chunk, :],
                )
                for j in range(s_chunk):
                    s = 1 + c * s_chunk + j
                    dps.append(dot_accum(nt[:, j, :], a_tile, dot_all[:, s : s + 1]))
                    nps.append(sq_accum(nt[:, j, :], ns2_all[:, s : s + 1]))
            dots.append(dot_all)
            ns2s.append(ns2_all)
            na2s.append(na2)
            dot_prods.append(dps)
            ns2_prods.append(nps)

        # ---- softmax epilogue on [128, s_tot] tiles ----
        loss_cols = sm.tile([128, n_groups], F32, tag="loss_cols")
        nss, nas = [], []
        for g in range(n_groups):  # all Sqrt together (one ACT table set)
            ns_ = sm.tile([128, s_tot], F32, tag=f"ns{g}")
            i_ns = nc.scalar.activation(out=ns_[:], in_=ns2s[g][:], func=AF.Sqrt)
            for p in ns2_prods[g]:
                add_dep_helper(i_ns.ins, p.ins, reason="accum ns2 -> sqrt")
            na_ = sm.tile([128, 1], F32, tag=f"na{g}")
            i_na = nc.scalar.activation(out=na_[:], in_=na2s[g][:], func=AF.Sqrt)
            add_dep_helper(i_na.ins, na2_prods[g].ins, reason="accum na2 -> sqrt")
            nss.append(ns_)
            nas.append(na_)
        sims = []
        for g in range(n_groups):
            ns_, na_ = nss[g], nas[g]
            nc.vector.tensor_scalar_max(ns_[:], ns_[:], EPS)
            nc.vector.tensor_scalar_max(na_[:], na_[:], EPS)
            denom = sm.tile([128, s_tot], F32, tag=f"den{g}")
            nc.vector.tensor_scalar_mul(denom[:], ns_[:], na_[:])
            inv = sm.tile([128, s_tot], F32, tag=f"inv{g}")
            nc.vector.reciprocal(out=inv[:], in_=denom[:])
            sim = sm.tile([128, s_tot], F32, tag=f"sim{g}")
            i_sim = nc.vector.tensor_mul(sim[:], dots[g][:], inv[:])
            for p in dot_prods[g]:
                add_dep_helper(i_sim.ins, p.ins, reason="accum dot -> sim")
            sims.append(sim)
        for g in range(n_groups):  # Exp and Ln share a table set
            # |sim| <= 1, so exp never overflows: no max-subtraction needed
            e = sm.tile([128, s_tot], F32, tag=f"e{g}")
            sumexp = sm.tile([128, 1], F32, tag=f"se{g}")
            i_exp = nc.scalar.activation(
                out=e[:], in_=sims[g][:], func=AF.Exp, accum_out=sumexp[:]
            )
            lse = sm.tile([128, 1], F32, tag=f"lse{g}")
            i_ln = nc.scalar.activation(out=lse[:], in_=sumexp[:], func=AF.Ln)
            add_dep_helper(i_ln.ins, i_exp.ins, reason="accum sumexp -> ln")
            nc.vector.tensor_sub(loss_cols[:, g : g + 1], lse[:], sims[g][:, 0:1])
        nc.sync.dma_start(out=out[:], in_=loss_cols[:])
    nc.finalize()
    return nc


_NC_CACHE = None


def _get_nc() -> bass.Bass:
    global _NC_CACHE
    if _NC_CACHE is None:
        _NC_CACHE = _build_nc()
    return _NC_CACHE


def _make_in_maps(anchor, positives, negatives):
    anchor = np.asarray(anchor)
    positives = np.asarray(positives)
    negatives = np.asarray(negatives)
    in_maps = []
    for i in range(N_CORES):
        sl = slice(i * B_LOC, (i + 1) * B_LOC)
        in_maps.append(
            {
                "anchor": np.ascontiguousarray(anchor[sl, 0, :], dtype=np.float32),
                "pos": np.ascontiguousarray(positives[sl, -1, :], dtype=np.float32),
                "neg": np.ascontiguousarray(negatives[sl], dtype=np.float32),
            }
        )
    return in_maps


def _reduce_results(results):
    total = 0.0
    for r in results:
        total += float(np.asarray(r["loss_cols"], dtype=np.float64).sum())
    return np.array(total / B, dtype=np.float32)


def run_sharded(anchor, positives, negatives, **spmd_kwargs):
    """Run on 8 cores; returns (loss_scalar, BassKernelResults)."""
    nc = _get_nc()
    in_maps = _make_in_maps(anchor, positives, negatives)
    res = run_bass_kernel_spmd(nc, in_maps, core_ids=list(range(N_CORES)), **spmd_kwargs)
    return _reduce_results(res.results), res


def kernel(anchor, positives, negatives):
    loss, _ = run_sharded(anchor, positives, negatives)
    return loss
